# revision 28
# baseline (speedup 1.0000x reference)
"""CST airfoil decoder kernel for Trainium2 (Bass/Tile), 8-core data parallel.

Problem (hardcoded): z (4096, 18) f32, x_coords (4096, 2048) f32
-> out (4096, 4096) f32 with out[:, 0::2] = x_coords, out[:, 1::2] = y.

Approach: the per-row curves y_L(x), y_U(x) are analytic in s = sqrt(x), so
the host fits each row's lower curve Phi_L and upper-minus-lower residual
Phi_D as degree-8 polynomials in u = 2*sqrt(x) - 1 (density-weighted LS on a
grid; fp16 coefficients; rel err ~1e-2, well under the 2e-2 gate). On device:

  u       = 2*sqrt(x) - 1                  (ACT sqrt, DVE affine)
  basis   = {1, u, u2, ..., u8}            (ACT squares + DVE odd products)
  Phi_L   = sum_k cL_k * u^k  -> PSUM      (PE diag-matmul accumulation)
  Phi_D   = sum_k d_k  * u^k  -> PSUM      (PE)
  m       = is_upper mask from prefix-min scan vs row min (DVE)
  y       = Phi_L + m * Phi_D              (DVE psum-mult, Pool psum-add)

The per-row coefficients ride in as host-built diagonal stationaries
(fp16 [128,128] per coefficient) so one matmul applies one coefficient
column to one basis tensor, accumulating in PSUM. PSUM is processed in
half-tiles [128, 1024] so the two accumulators double-buffer in 8 banks.

Sharding: pure data parallel over batch, 512 rows per core.
"""

import math

import numpy as np

import concourse.bacc as bacc
import concourse.bass as bass
import concourse.mybir as mybir
from concourse.bass_utils import run_bass_kernel_spmd
from concourse.tile import TileContext

B, NZ = 4096, 18
N = 2048
N_CORES = 8
ROWS_PER_CORE = B // N_CORES          # 512
P = 128
TILES = ROWS_PER_CORE // P            # 4
KS = (0, 1, 2, 3, 4, 5, 6, 8)         # basis powers u^k used by both fits
KSNZ = KS[1:]                         # nonconstant powers ride on PE
NK = len(KSNZ)                        # 7 matmul terms per side
NCOEF = 2 * NK                        # L + D diag sets (constants ride DVE)
H = N // 2                            # half-tile width (psum double buffer)
GRID = 192                            # host fit grid
WPOW = 0.5                            # fit weight s**WPOW

F32 = mybir.dt.float32
F16 = mybir.dt.float16
Alu = mybir.AluOpType
Act = mybir.ActivationFunctionType


def _f16(a: np.ndarray) -> np.ndarray:
    return np.asarray(a, dtype=np.float16).astype(np.float32)


def _y_side(z64: np.ndarray, xg: np.ndarray, upper: bool) -> np.ndarray:
    """Exact reference curve per row on grid xg (G,) -> (B, G)."""
    n = 8
    lower = z64[:, :n]
    upper_c = z64[:, n:2 * n]
    le = z64[:, 16][:, None]
    te = z64[:, 17][:, None]
    xc = np.clip(xg, 1e-8, 1 - 1e-8)
    C = xc ** 0.5 * (1.0 - xc)
    binom = np.array([math.comb(7, k) for k in range(n)], dtype=np.float64)
    k = np.arange(n)
    S = binom * xg[None, :, None] ** k * (1 - xg[None, :, None]) ** (7 - k)
    Pp = np.einsum('bgk,bk->bg', S, upper_c if upper else lower)
    y = C[None, :] * Pp + le * xg[None, :] * (1 - xg[None, :]) ** 8.5
    half = xg[None, :] * te * 0.5
    return y + (half if upper else -half)


def _host_coeffs(z: np.ndarray) -> np.ndarray:
    """Fit Phi_L, Phi_D per row; return (B, NCOEF) bf16-rounded f32."""
    z64 = z.astype(np.float64)
    sg = (np.arange(GRID) + 0.5) / GRID
    ug = 2 * sg - 1
    W = sg ** WPOW
    V = ug[:, None] ** np.array(KS)           # (G, NK)
    VW = V * W[:, None]
    G = VW.T @ VW
    nf = len(KS)
    A = np.linalg.solve(G + 1e-11 * np.trace(G) / nf * np.eye(nf), VW.T)
    yL = _y_side(z64, sg ** 2, False)          # (B, G)
    yU = _y_side(z64, sg ** 2, True)
    aL = _f16((A @ (W[:, None] * yL.T)).T).astype(np.float64)
    resU = yU - aL @ V.T
    aD = _f16((A @ (W[:, None] * resU.T)).T)
    # [aL_1.. aL_8 | aD_1.. aD_8 | aL_0 | aD_0]: nonconstant coeffs for the
    # PE diags, constants for the DVE tail's scalar slots.
    return np.concatenate([aL[:, 1:].astype(np.float32), aD[:, 1:],
                           aL[:, 0:1].astype(np.float32), aD[:, 0:1]],
                          axis=1)


def _host_diags(coefs: np.ndarray) -> np.ndarray:
    """Per-core diag stationaries.

    coefs: (ROWS_PER_CORE, NCOEF) f32 (bf16-valued). Returns uint16 bf16-bits
    array (P, TILES*NCOEF*P): partition c, free (t, j, q) holds
    coefs[t*P + c, j] iff q == c else 0.
    """
    out = np.zeros((P, TILES, NCOEF, P), dtype=np.uint16)
    bits = coefs[:, :NCOEF].astype(np.float16).view(np.uint16)
    idx = np.arange(P)
    for t in range(TILES):
        for j in range(NCOEF):
            out[idx, t, j, idx] = bits[t * P:(t + 1) * P, j]
    return out.reshape(P, TILES * NCOEF * P)


def _build_program() -> bass.Bass:
    nc = bacc.Bacc("TRN2", debug=False, num_devices=N_CORES,
                   enable_partition_id=False)
    x_d = nc.dram_tensor("x", (ROWS_PER_CORE, N), F32, kind="ExternalInput")
    sc_d = nc.dram_tensor("sc", (ROWS_PER_CORE, 2), F32, kind="ExternalInput")
    dg_d = nc.dram_tensor("diag", (P, TILES * NCOEF * P), F16,
                          kind="ExternalInput")
    out_d = nc.dram_tensor("out", (ROWS_PER_CORE, 2 * N), F32,
                           kind="ExternalOutput")

    with TileContext(nc) as tc:
        with tc.tile_pool(name="io", bufs=1) as io_pool, \
             tc.tile_pool(name="scr", bufs=1) as scr, \
             tc.psum_pool(name="ps", bufs=1) as pp:
            ones = scr.tile([P, N], F16, tag="ones", name="ones")
            nc.vector.memset(ones[:, :], 1.0)
            inclp = scr.tile([P, N + 8], F32, tag="inclp", name="inclp")
            nc.gpsimd.memset(inclp[:, 0:1], 2.0)
            diag = scr.tile([P, TILES * NCOEF * P], F16, tag="diag",
                            name="diag")

            # PE warm-up: dummy matmuls on `ones` (no DMA dependency) keep PE
            # continuously busy through the startup window, so the p-state
            # ramp (full clock after 3us busy) completes before real work.
            warm = pp.tile([P, 512], F32, tag="psL", name="warm", bufs=4)
            for _ in range(8):
                nc.tensor.matmul(out=warm[:, :], lhsT=ones[:, 0:P],
                                 rhs=ones[:, 0:512], start=True, stop=True)

            def emit_head(t):
                """DMAs + basis + scan/mask + xcopy for tile t; returns its
                tile handles."""
                r0 = t * P
                x = io_pool.tile([P, N], F32, tag="x", bufs=3, name="x")
                out = io_pool.tile([P, 2 * N], F32, tag="out", bufs=2,
                                   name="out")
                if t == 0:
                    # tile-0: first x quarter, then diag (PE start), then
                    # the rest — the chain starts after just 512 columns.
                    nc.sync.dma_start(out=x[:, 0:512],
                                      in_=x_d.ap()[r0:r0 + P, 0:512])
                    nc.sync.dma_start(out=diag[:, 0:NCOEF * P],
                                      in_=dg_d.ap()[:, 0:NCOEF * P])
                    for i in range(1, 4):
                        nc.sync.dma_start(
                            out=x[:, i * 512:(i + 1) * 512],
                            in_=x_d.ap()[r0:r0 + P, i * 512:(i + 1) * 512])
                    for t2 in range(1, TILES):
                        c0 = t2 * NCOEF * P
                        nc.sync.dma_start(out=diag[:, c0:c0 + NCOEF * P],
                                          in_=dg_d.ap()[:, c0:c0 + NCOEF * P])
                else:
                    nc.sync.dma_start(out=x[:, 0:H],
                                      in_=x_d.ap()[r0:r0 + P, 0:H])
                    nc.sync.dma_start(out=x[:, H:N],
                                      in_=x_d.ap()[r0:r0 + P, H:N])

                sc = scr.tile([P, 2], F32, tag="sc", name="sc", bufs=2)
                nc.sync.dma_start(out=sc[:, :], in_=sc_d.ap()[r0:r0 + P, :])
                s = scr.tile([P, N], F16, tag="s", name="s", bufs=2)
                u = scr.tile([P, N], F16, tag="u", name="u", bufs=2)
                u2 = scr.tile([P, N], F16, tag="u2", name="u2", bufs=2)
                u3 = scr.tile([P, N], F16, tag="u3", name="u3", bufs=2)
                u4 = scr.tile([P, N], F16, tag="u4", name="u4", bufs=2)
                u5 = scr.tile([P, N], F16, tag="u5", name="u5", bufs=2)
                u6 = scr.tile([P, N], F16, tag="u6", name="u6", bufs=2)
                u8 = scr.tile([P, N], F16, tag="u8", name="u8", bufs=2)
                mk = scr.tile([P, N], F16, tag="mk", name="mk", bufs=2)

                # Tile 0 builds the basis per 512-chunk so the chain latency
                # (which gates PE's first accumulations) is quartered.
                for bh in (tuple(slice(i * 512, (i + 1) * 512)
                                 for i in range(4)) if t == 0
                           else (slice(0, N),)):
                    nc.scalar.activation(out=s[:, bh], in_=x[:, bh],
                                         func=Act.Sqrt)
                    nc.vector.tensor_scalar(out=u[:, bh], in0=s[:, bh],
                                            scalar1=2.0, scalar2=-1.0,
                                            op0=Alu.mult, op1=Alu.add)
                    nc.scalar.activation(out=u2[:, bh], in_=u[:, bh],
                                         func=Act.Square)
                    nc.vector.tensor_mul(out=u3[:, bh], in0=u[:, bh],
                                         in1=u2[:, bh])
                    nc.scalar.activation(out=u4[:, bh], in_=u2[:, bh],
                                         func=Act.Square)
                    nc.gpsimd.tensor_tensor(out=u5[:, bh], in0=u[:, bh],
                                            in1=u4[:, bh], op=Alu.mult)
                    nc.scalar.activation(out=u6[:, bh], in_=u3[:, bh],
                                         func=Act.Square)
                    nc.scalar.activation(out=u8[:, bh], in_=u4[:, bh],
                                         func=Act.Square)

                # mask: exclusive prefix-min > row min (col 0 seeded 2.0 is
                # the exclusive-scan seed)
                nc.vector.tensor_tensor_scan(
                    out=inclp[:, 1:N + 1], data0=x[:, :], data1=x[:, :],
                    initial=2.0, op0=Alu.min, op1=Alu.min)
                nc.gpsimd.tensor_scalar(
                    out=mk[:, :], in0=inclp[:, 0:N],
                    scalar1=inclp[:, N:N + 1], scalar2=None, op0=Alu.is_gt)

                out3 = out[:, :].rearrange("p (n two) -> p n two", two=2)
                nc.gpsimd.tensor_copy(out3[:, :, 0:1], x[:, :])
                return dict(x=x, out=out, out3=out3, mk=mk, sc=sc,
                            basis=[u, u2, u3, u4, u5, u6, u8])

            heads = emit_head(0)
            for t in range(TILES):
                r0 = t * P
                cur = heads
                dbase = t * NCOEF * P
                out3, mk, basis, out = (cur["out3"], cur["mk"],
                                        cur["basis"], cur["out"])
                sc = cur["sc"]
                for ci, (h, c) in enumerate((h, c) for h in range(2)
                                            for c in range(H // 512)):
                    # software pipeline: emit the next tile's head midway
                    # through this tile's chunk stream so its DVE/ACT chain
                    # overlaps this tile's PE work.
                    if ci == 2 and t + 1 < TILES:
                        heads = emit_head(t + 1)
                    cs = slice(h * H + c * 512, h * H + (c + 1) * 512)
                    psL = pp.tile([P, 512], F32, tag="psL", name="psL",
                                  bufs=4)
                    psD = pp.tile([P, 512], F32, tag="psD", name="psD",
                                  bufs=4)
                    # D first: its DVE consumer (mask-mult) overlaps the
                    # L accumulation that follows.
                    for k in range(NK):
                        dk = slice(dbase + (NK + k) * P,
                                   dbase + (NK + k + 1) * P)
                        nc.tensor.matmul(out=psD[:, :], lhsT=diag[:, dk],
                                         rhs=basis[k][:, cs],
                                         start=(k == 0), stop=(k == NK - 1))
                    for k in range(NK):
                        dk = slice(dbase + k * P, dbase + (k + 1) * P)
                        nc.tensor.matmul(out=psL[:, :], lhsT=diag[:, dk],
                                         rhs=basis[k][:, cs],
                                         start=(k == 0), stop=(k == NK - 1))
                    # y = Phi_L + m * Phi_D into the odd out columns
                    md = scr.tile([P, 512], F16, tag="md", name="md", bufs=4)
                    # md = (Phi_D' + d0) * m  -- folds the D constant term
                    nc.vector.scalar_tensor_tensor(
                        out=md[:, :], in0=psD[:, :], scalar=sc[:, 1:2],
                        in1=mk[:, cs], op0=Alu.add, op1=Alu.mult)
                    final = (t == TILES - 1 and h == 1 and c == H // 512 - 1)
                    # final chunk: 256-col pieces so the last store pipeline
                    # (add -> DMA) overlaps itself
                    for q in range(2 if final else 1):
                        qn = 256 if final else 512
                        q0 = h * H + c * 512 + q * qn
                        # y = (Phi_L' + cL0) + md -- folds the L constant
                        nc.vector.scalar_tensor_tensor(
                            out=out3[:, q0:q0 + qn, 1:2],
                            in0=psL[:, q * qn:(q + 1) * qn if final else 512],
                            scalar=sc[:, 0:1],
                            in1=md[:, q * qn:(q + 1) * qn if final else 512],
                            op0=Alu.add, op1=Alu.add)
                        if final:
                            nc.sync.dma_start(
                                out=out_d.ap()[r0:r0 + P,
                                               2 * q0:2 * (q0 + qn)],
                                in_=out[:, 2 * q0:2 * (q0 + qn)])
                    if t == TILES - 1 and h == 1 and not final:
                        # last half, non-final chunk: per-chunk DMA
                        o0 = 2 * (h * H + c * 512)
                        nc.sync.dma_start(
                            out=out_d.ap()[r0:r0 + P, o0:o0 + 1024],
                            in_=out[:, o0:o0 + 1024])
                    elif not final and c == H // 512 - 1:
                        nc.sync.dma_start(
                            out=out_d.ap()[r0:r0 + P,
                                           2 * h * H:2 * (h + 1) * H],
                            in_=out[:, 2 * h * H:2 * (h + 1) * H])
    nc.compile()
    return nc


_PROGRAM: bass.Bass | None = None


def _program() -> bass.Bass:
    global _PROGRAM
    if _PROGRAM is None:
        _PROGRAM = _build_program()
    return _PROGRAM


def kernel(z, x_coords, _run_kwargs: dict | None = None):
    z = np.asarray(z, dtype=np.float32)
    x_coords = np.ascontiguousarray(np.asarray(x_coords, dtype=np.float32))
    assert z.shape == (B, NZ) and x_coords.shape == (B, N)

    coefs = _host_coeffs(z)
    in_maps = []
    for c in range(N_CORES):
        r = slice(c * ROWS_PER_CORE, (c + 1) * ROWS_PER_CORE)
        diags = _host_diags(coefs[r])
        in_maps.append({"x": np.ascontiguousarray(x_coords[r]),
                        "sc": np.ascontiguousarray(coefs[r, NCOEF:]),
                        "diag": diags})

    res = run_bass_kernel_spmd(_program(), in_maps,
                               core_ids=list(range(N_CORES)),
                               **(_run_kwargs or {}))
    out = np.concatenate([r["out"] for r in res.results], axis=0)
    if _run_kwargs:
        kernel.last_results = res
    return out


# revision 29
# speedup vs baseline: 1.2887x; 1.2887x over previous
"""CST airfoil decoder kernel for Trainium2 (Bass/Tile), 8-core data parallel.

Problem (hardcoded): z (4096, 18) f32, x_coords (4096, 2048) f32
-> out (4096, 4096) f32 with out[:, 0::2] = x_coords, out[:, 1::2] = y.

Approach: the per-row curves y_L(x), y_U(x) are analytic in s = sqrt(x), so
the host fits each row's lower curve Phi_L and upper-minus-lower residual
Phi_D as degree-8 polynomials in u = 2*sqrt(x) - 1 (density-weighted LS on a
grid; fp16 coefficients; rel err ~1e-2, well under the 2e-2 gate). On device:

  u       = 2*sqrt(x) - 1                  (ACT sqrt, DVE affine)
  basis   = {1, u, u2, ..., u8}            (ACT squares + DVE odd products)
  Phi_L   = sum_k cL_k * u^k  -> PSUM      (PE diag-matmul accumulation)
  Phi_D   = sum_k d_k  * u^k  -> PSUM      (PE)
  m       = is_upper mask from prefix-min scan vs row min (DVE)
  y       = Phi_L + m * Phi_D              (DVE psum-mult, Pool psum-add)

The per-row coefficients ride in as host-built diagonal stationaries
(fp16 [128,128] per coefficient) so one matmul applies one coefficient
column to one basis tensor, accumulating in PSUM. PSUM is processed in
half-tiles [128, 1024] so the two accumulators double-buffer in 8 banks.

Sharding: pure data parallel over batch, 512 rows per core.
"""

import math

import numpy as np

import concourse.bacc as bacc
import concourse.bass as bass
import concourse.mybir as mybir
from concourse.bass_utils import run_bass_kernel_spmd
from concourse.tile import TileContext

B, NZ = 4096, 18
N = 2048
N_CORES = 8
ROWS_PER_CORE = B // N_CORES          # 512
P = 128
TILES = ROWS_PER_CORE // P            # 4
KS = (0, 1, 2, 3, 4, 5, 6, 8)         # basis powers u^k used by both fits
KSNZ = KS[1:]                         # nonconstant powers ride on PE
NK = len(KSNZ)                        # 7 matmul terms per side
NCOEF = 2 * NK                        # L + D diag sets (constants ride DVE)
H = N // 2                            # half-tile width (psum double buffer)
GRID = 192                            # host fit grid
WPOW = 0.5                            # fit weight s**WPOW

F32 = mybir.dt.float32
F16 = mybir.dt.float16
Alu = mybir.AluOpType
Act = mybir.ActivationFunctionType


def _f16(a: np.ndarray) -> np.ndarray:
    return np.asarray(a, dtype=np.float16).astype(np.float32)


def _y_side(z64: np.ndarray, xg: np.ndarray, upper: bool) -> np.ndarray:
    """Exact reference curve per row on grid xg (G,) -> (B, G)."""
    n = 8
    lower = z64[:, :n]
    upper_c = z64[:, n:2 * n]
    le = z64[:, 16][:, None]
    te = z64[:, 17][:, None]
    xc = np.clip(xg, 1e-8, 1 - 1e-8)
    C = xc ** 0.5 * (1.0 - xc)
    binom = np.array([math.comb(7, k) for k in range(n)], dtype=np.float64)
    k = np.arange(n)
    S = binom * xg[None, :, None] ** k * (1 - xg[None, :, None]) ** (7 - k)
    Pp = np.einsum('bgk,bk->bg', S, upper_c if upper else lower)
    y = C[None, :] * Pp + le * xg[None, :] * (1 - xg[None, :]) ** 8.5
    half = xg[None, :] * te * 0.5
    return y + (half if upper else -half)


def _host_coeffs(z: np.ndarray) -> np.ndarray:
    """Fit Phi_L, Phi_D per row; return (B, NCOEF) bf16-rounded f32."""
    z64 = z.astype(np.float64)
    sg = (np.arange(GRID) + 0.5) / GRID
    ug = 2 * sg - 1
    W = sg ** WPOW
    V = ug[:, None] ** np.array(KS)           # (G, NK)
    VW = V * W[:, None]
    G = VW.T @ VW
    nf = len(KS)
    A = np.linalg.solve(G + 1e-11 * np.trace(G) / nf * np.eye(nf), VW.T)
    yL = _y_side(z64, sg ** 2, False)          # (B, G)
    yU = _y_side(z64, sg ** 2, True)
    aL = _f16((A @ (W[:, None] * yL.T)).T).astype(np.float64)
    resU = yU - aL @ V.T
    aD = _f16((A @ (W[:, None] * resU.T)).T)
    # [aL_1.. aL_8 | aD_1.. aD_8 | aL_0 | aD_0]: nonconstant coeffs for the
    # PE diags, constants for the DVE tail's scalar slots.
    return np.concatenate([aL[:, 1:].astype(np.float32), aD[:, 1:],
                           aL[:, 0:1].astype(np.float32), aD[:, 0:1]],
                          axis=1)


def _host_diags(coefs: np.ndarray) -> np.ndarray:
    """Per-core diag stationaries.

    coefs: (ROWS_PER_CORE, NCOEF) f32 (bf16-valued). Returns uint16 bf16-bits
    array (P, TILES*NCOEF*P): partition c, free (t, j, q) holds
    coefs[t*P + c, j] iff q == c else 0.
    """
    out = np.zeros((P, TILES, NCOEF, P), dtype=np.uint16)
    bits = coefs[:, :NCOEF].astype(np.float16).view(np.uint16)
    idx = np.arange(P)
    for t in range(TILES):
        for j in range(NCOEF):
            out[idx, t, j, idx] = bits[t * P:(t + 1) * P, j]
    return out.reshape(P, TILES * NCOEF * P)


def _build_program() -> bass.Bass:
    nc = bacc.Bacc("TRN2", debug=False, num_devices=N_CORES,
                   enable_partition_id=False)
    x_d = nc.dram_tensor("x", (ROWS_PER_CORE, N), F32, kind="ExternalInput")
    sc_d = nc.dram_tensor("sc", (ROWS_PER_CORE, 2), F32, kind="ExternalInput")
    dg_d = nc.dram_tensor("diag", (P, TILES * NCOEF * P), F16,
                          kind="ExternalInput")
    out_d = nc.dram_tensor("out", (ROWS_PER_CORE, 2 * N), F32,
                           kind="ExternalOutput")

    with TileContext(nc) as tc:
        with tc.tile_pool(name="io", bufs=1) as io_pool, \
             tc.tile_pool(name="scr", bufs=1) as scr, \
             tc.psum_pool(name="ps", bufs=1) as pp:
            ones = scr.tile([P, N], F16, tag="ones", name="ones")
            nc.vector.memset(ones[:, :], 1.0)
            inclp = scr.tile([P, N + 8], F32, tag="inclp", name="inclp")
            nc.gpsimd.memset(inclp[:, 0:1], 2.0)
            diag = scr.tile([P, TILES * NCOEF * P], F16, tag="diag",
                            name="diag")

            # PE warm-up: dummy matmuls on `ones` (no DMA dependency) keep PE
            # continuously busy through the startup window, so the p-state
            # ramp (full clock after 3us busy) completes before real work.
            warm = pp.tile([P, 512], F32, tag="psL", name="warm", bufs=4)
            for _ in range(8):
                nc.tensor.matmul(out=warm[:, :], lhsT=ones[:, 0:P],
                                 rhs=ones[:, 0:512], start=True, stop=True)

            def emit_head(t):
                """DMAs + basis + scan/mask + xcopy for tile t; returns its
                tile handles."""
                r0 = t * P
                x = io_pool.tile([P, N], F32, tag="x", bufs=3, name="x")
                out = io_pool.tile([P, 2 * N], F32, tag="out", bufs=2,
                                   name="out")
                if t == 0:
                    # tile-0: first x quarter, then diag (PE start), then
                    # the rest — the chain starts after just 512 columns.
                    nc.sync.dma_start(out=x[:, 0:512],
                                      in_=x_d.ap()[r0:r0 + P, 0:512])
                    nc.sync.dma_start(out=diag[:, 0:NCOEF * P],
                                      in_=dg_d.ap()[:, 0:NCOEF * P])
                    for i in range(1, 4):
                        nc.sync.dma_start(
                            out=x[:, i * 512:(i + 1) * 512],
                            in_=x_d.ap()[r0:r0 + P, i * 512:(i + 1) * 512])
                    for t2 in range(1, TILES):
                        c0 = t2 * NCOEF * P
                        nc.sync.dma_start(out=diag[:, c0:c0 + NCOEF * P],
                                          in_=dg_d.ap()[:, c0:c0 + NCOEF * P])
                else:
                    nc.sync.dma_start(out=x[:, 0:H],
                                      in_=x_d.ap()[r0:r0 + P, 0:H])
                    nc.sync.dma_start(out=x[:, H:N],
                                      in_=x_d.ap()[r0:r0 + P, H:N])

                sc = scr.tile([P, 2], F32, tag="sc", name="sc", bufs=2)
                nc.sync.dma_start(out=sc[:, :], in_=sc_d.ap()[r0:r0 + P, :])
                s = scr.tile([P, N], F16, tag="s", name="s", bufs=2)
                u = scr.tile([P, N], F16, tag="u", name="u", bufs=2)
                u2 = scr.tile([P, N], F16, tag="u2", name="u2", bufs=2)
                u3 = scr.tile([P, N], F16, tag="u3", name="u3", bufs=2)
                u4 = scr.tile([P, N], F16, tag="u4", name="u4", bufs=2)
                u5 = scr.tile([P, N], F16, tag="u5", name="u5", bufs=2)
                u6 = scr.tile([P, N], F16, tag="u6", name="u6", bufs=2)
                u8 = scr.tile([P, N], F16, tag="u8", name="u8", bufs=2)
                mk = scr.tile([P, N], F16, tag="mk", name="mk", bufs=2)

                # Tile 0 builds the basis per 512-chunk so the chain latency
                # (which gates PE's first accumulations) is quartered.
                for bh in (tuple(slice(i * 512, (i + 1) * 512)
                                 for i in range(4)) if t == 0
                           else (slice(0, N),)):
                    nc.scalar.activation(out=s[:, bh], in_=x[:, bh],
                                         func=Act.Sqrt)
                    nc.vector.tensor_scalar(out=u[:, bh], in0=s[:, bh],
                                            scalar1=2.0, scalar2=-1.0,
                                            op0=Alu.mult, op1=Alu.add)
                    nc.scalar.activation(out=u2[:, bh], in_=u[:, bh],
                                         func=Act.Square)
                    nc.vector.tensor_mul(out=u3[:, bh], in0=u[:, bh],
                                         in1=u2[:, bh])
                    nc.scalar.activation(out=u4[:, bh], in_=u2[:, bh],
                                         func=Act.Square)
                    nc.vector.tensor_mul(out=u5[:, bh], in0=u[:, bh],
                                         in1=u4[:, bh])
                    nc.scalar.activation(out=u6[:, bh], in_=u3[:, bh],
                                         func=Act.Square)
                    nc.scalar.activation(out=u8[:, bh], in_=u4[:, bh],
                                         func=Act.Square)

                # mask: exclusive prefix-min > row min (col 0 seeded 2.0 is
                # the exclusive-scan seed)
                nc.vector.tensor_tensor_scan(
                    out=inclp[:, 1:N + 1], data0=x[:, :], data1=x[:, :],
                    initial=2.0, op0=Alu.min, op1=Alu.min)
                nc.vector.tensor_scalar(
                    out=mk[:, :], in0=inclp[:, 0:N],
                    scalar1=inclp[:, N:N + 1], scalar2=None, op0=Alu.is_gt)

                out3 = out[:, :].rearrange("p (n two) -> p n two", two=2)
                nc.gpsimd.tensor_copy(out3[:, :, 0:1], x[:, :])
                return dict(x=x, out=out, out3=out3, mk=mk, sc=sc,
                            basis=[u, u2, u3, u4, u5, u6, u8])

            heads = emit_head(0)
            for t in range(TILES):
                r0 = t * P
                cur = heads
                dbase = t * NCOEF * P
                out3, mk, basis, out = (cur["out3"], cur["mk"],
                                        cur["basis"], cur["out"])
                sc = cur["sc"]
                for ci, (h, c) in enumerate((h, c) for h in range(2)
                                            for c in range(H // 512)):
                    # software pipeline: emit the next tile's head midway
                    # through this tile's chunk stream so its DVE/ACT chain
                    # overlaps this tile's PE work.
                    if ci == 2 and t + 1 < TILES:
                        heads = emit_head(t + 1)
                    cs = slice(h * H + c * 512, h * H + (c + 1) * 512)
                    psL = pp.tile([P, 512], F32, tag="psL", name="psL",
                                  bufs=4)
                    psD = pp.tile([P, 512], F32, tag="psD", name="psD",
                                  bufs=4)
                    # D first: its DVE consumer (mask-mult) overlaps the
                    # L accumulation that follows.
                    for k in range(NK):
                        dk = slice(dbase + (NK + k) * P,
                                   dbase + (NK + k + 1) * P)
                        nc.tensor.matmul(out=psD[:, :], lhsT=diag[:, dk],
                                         rhs=basis[k][:, cs],
                                         start=(k == 0), stop=(k == NK - 1))
                    for k in range(NK):
                        dk = slice(dbase + k * P, dbase + (k + 1) * P)
                        nc.tensor.matmul(out=psL[:, :], lhsT=diag[:, dk],
                                         rhs=basis[k][:, cs],
                                         start=(k == 0), stop=(k == NK - 1))
                    # y = Phi_L + m * Phi_D into the odd out columns
                    md = scr.tile([P, 512], F16, tag="md", name="md", bufs=4)
                    # md = (Phi_D' + d0) * m  -- folds the D constant term
                    nc.vector.scalar_tensor_tensor(
                        out=md[:, :], in0=psD[:, :], scalar=sc[:, 1:2],
                        in1=mk[:, cs], op0=Alu.add, op1=Alu.mult)
                    final = (t == TILES - 1 and h == 1 and c == H // 512 - 1)
                    # final chunk: 256-col pieces so the last store pipeline
                    # (add -> DMA) overlaps itself
                    for q in range(2 if final else 1):
                        qn = 256 if final else 512
                        q0 = h * H + c * 512 + q * qn
                        # y = (Phi_L' + cL0) + md -- folds the L constant
                        nc.vector.scalar_tensor_tensor(
                            out=out3[:, q0:q0 + qn, 1:2],
                            in0=psL[:, q * qn:(q + 1) * qn if final else 512],
                            scalar=sc[:, 0:1],
                            in1=md[:, q * qn:(q + 1) * qn if final else 512],
                            op0=Alu.add, op1=Alu.add)
                        if final:
                            nc.sync.dma_start(
                                out=out_d.ap()[r0:r0 + P,
                                               2 * q0:2 * (q0 + qn)],
                                in_=out[:, 2 * q0:2 * (q0 + qn)])
                    if t == TILES - 1 and h == 1 and not final:
                        # last half, non-final chunk: per-chunk DMA
                        o0 = 2 * (h * H + c * 512)
                        nc.sync.dma_start(
                            out=out_d.ap()[r0:r0 + P, o0:o0 + 1024],
                            in_=out[:, o0:o0 + 1024])
                    elif not final and c == H // 512 - 1:
                        nc.sync.dma_start(
                            out=out_d.ap()[r0:r0 + P,
                                           2 * h * H:2 * (h + 1) * H],
                            in_=out[:, 2 * h * H:2 * (h + 1) * H])
    nc.compile()
    return nc


_PROGRAM: bass.Bass | None = None


def _program() -> bass.Bass:
    global _PROGRAM
    if _PROGRAM is None:
        _PROGRAM = _build_program()
    return _PROGRAM


def kernel(z, x_coords, _run_kwargs: dict | None = None):
    z = np.asarray(z, dtype=np.float32)
    x_coords = np.ascontiguousarray(np.asarray(x_coords, dtype=np.float32))
    assert z.shape == (B, NZ) and x_coords.shape == (B, N)

    coefs = _host_coeffs(z)
    in_maps = []
    for c in range(N_CORES):
        r = slice(c * ROWS_PER_CORE, (c + 1) * ROWS_PER_CORE)
        diags = _host_diags(coefs[r])
        in_maps.append({"x": np.ascontiguousarray(x_coords[r]),
                        "sc": np.ascontiguousarray(coefs[r, NCOEF:]),
                        "diag": diags})

    res = run_bass_kernel_spmd(_program(), in_maps,
                               core_ids=list(range(N_CORES)),
                               **(_run_kwargs or {}))
    out = np.concatenate([r["out"] for r in res.results], axis=0)
    if _run_kwargs:
        kernel.last_results = res
    return out
